# revision 39
# baseline (speedup 1.0000x reference)
"""Trainium2 Bass kernel for nn_FourDirectionalCrossModalScan.

SPMD over 8 NeuronCores, one batch element per core PAIR: core 2b runs the
HORIZONTAL bicms of image b, core 2b+1 the VERTICAL bicms. The program is
identical on every core — direction is selected purely by inputs:
  - dtok: tokens pre-interleaved (sem,inst) in this core's scan order
  - weight slots 0/1 hold this core's (fwd, bwd) stacks (host-reordered)
  - csum_f/csum_b: real cumsum-select for h-cores; +1e4 on v-cores, which
    drives the chunk-carry weight exp(csum@dtA) to 0 (dtA<0 strictly), so
    the two 64-token chunks of a tile stay independent sequences
After the gated bicms, each core's result tiles are written (scaled by a
per-core {0,1} mask) to a DRAM bounce buffer; a pairwise AllReduce(add)
leaves the v-core's results on both cores. Every core then merges
self-as-H with peer-as-V (a compile-time col-major->row-major access
pattern) and LayerNorms — correct on even cores, garbage-but-finite on odd
cores, and the host reads even cores only.

The Mamba scan uses a chunked (SSD) decomposition with chunk Q=64:
  P(t,d) = exp(cumsum_chunk(dt*A))   -- token-major via block-tri matmul + Exp
  v      = dt*silu(xin) / P
  SMT    = B_c^T C_c (per chunk, causal-masked)
  y(t,d) = P * [ SMT^T v  +  C^T H_prev ],   H = P_end * (B^T v)

Perf structure: the exponent path (dt -> dt*A -> cumsum -> exp) stays fp32
for precision; value-path tensors (xs, sz, u, v, P, 1/P, B^T, SMT, q*, yf)
are bf16 (2x-rate DVE, full-rate PE). The four yf transposes land in one
PSUM bank drained by one activation. Ops that are identities for this
problem's parameters (ln affine, Dskip, gate/merge biases, merge-LN
affine) are compiled out; kernel() verifies those assumptions on the
actual inputs and refuses to run on violation.
"""
import numpy as np
from contextlib import ExitStack

import concourse.bass as bass
import concourse.bacc as bacc
import concourse.tile as tile
import concourse.mybir as mybir

F32 = mybir.dt.float32
F32R = mybir.dt.float32r
BF16 = mybir.dt.bfloat16
AF = mybir.ActivationFunctionType
OP = mybir.AluOpType

DIM = 256
DI = 512
ST = 64
NL = 2
SH, SW = 32, 64
BATCH = 4
EPS = 1e-5
TG = 512
NT = 4
NTOK = 2 * SH * SW  # 4096 interleaved tokens per direction


class KC:
    def __init__(self, nc, tc, ctx):
        self.nc = nc
        self.tc = tc
        self.ctx = ctx
        self.pools = {}

    def pool(self, name, bufs, space="SBUF"):
        if name not in self.pools:
            self.pools[name] = self.ctx.enter_context(
                self.tc.tile_pool(name=name, bufs=bufs, space=space))
        return self.pools[name]

    def tf(self, shape=(128, TG)):
        return self.pool("tf", 7).tile(list(shape), F32, tag="tf", name="tf")

    def tb(self, shape=(128, TG)):
        return self.pool("tb", 12).tile(list(shape), BF16, tag="tb", name="tb")

    def tr(self, shape=(128, TG)):
        return self.pool("tr", 10).tile(list(shape), F32R, tag="tr", name="tr")

    def ts(self, shape, dtype=F32):
        return self.pool("ts", 9).tile(list(shape), dtype, tag="ts", name="ts")

    def th(self):
        return self.pool("th", 5).tile([64, DI], F32R, tag="th", name="th")

    def ps(self, shape=(128, TG), dtype=F32):
        return self.pool("psA", 6, space="PSUM").tile(list(shape), dtype, tag="ps", name="ps")

    def pss(self, shape=(128, 128), dtype=F32):
        return self.pool("psS", 2, space="PSUM").tile(list(shape), dtype, tag="pss", name="pss")

    def wt(self, tag, shape, dtype=F32R, big=False):
        return self.pool("wbig" if big else "wrest", 2 if big else 1).tile(
            list(shape), dtype, tag=tag, name=tag)

    def xt(self):
        return self.pool("xio", 10).tile([128, TG], F32R, tag="xc", name="xc")

    def yf_t(self):
        return self.pool("yfd", 2).tile([128, 4 * TG], F32R, tag="yfd", name="yfd")

    def mt(self):
        return self.pool("mio", 4).tile([128, TG], F32R, tag="mg", name="mg")


def emit_load_weights(C, D, si, l):
    nc = C.nc
    W = {}
    for name in ("Wx", "Wz", "Wdt"):
        W[name] = []
        for i in range(2):
            t = C.wt(f"{name}{i}", [128, DI], big=True)
            nc.sync.dma_start(t[:], D[name][si, l, i * 128:(i + 1) * 128, :])
            W[name].append(t)
    for name in ("WB", "WC"):
        W[name] = []
        for i in range(2):
            t = C.wt(f"{name}{i}", [128, ST])
            nc.sync.dma_start(t[:], D[name][si, l, i * 128:(i + 1) * 128, :])
            W[name].append(t)
    W["Wout"] = []
    for db in range(4):
        t = C.wt(f"Wout{db}", [128, DIM])
        nc.sync.dma_start(t[:], D["Wout"][si, l, db * 128:(db + 1) * 128, :])
        W["Wout"].append(t)
    W["dt_bias"] = C.wt("dtb", [1, DI])
    nc.sync.dma_start(W["dt_bias"][:], D["dt_bias"][si, l])
    t = C.wt("A_bc", [128, DI], F32)
    nc.sync.dma_start(t[:], D["A_bc"][si, l])
    W["A_bc"] = t
    return W


def emit_layer(C, xs, W, fwd):
    nc = C.nc

    # layernorm stats (feature-major; cross-partition sums via ones-matmul)
    S1 = C.ps()
    for i, x in enumerate(xs):
        nc.tensor.matmul(S1[:], C.ones128[:], x[:], start=(i == 0), stop=(i == 1))
    S2 = C.ps()
    for i, x in enumerate(xs):
        sq = C.tr()
        nc.scalar.activation(sq[:], x[:].bitcast(F32), AF.Square)
        nc.tensor.matmul(S2[:], C.ones128[:], sq[:], start=(i == 0), stop=(i == 1))
    mu = C.tf()
    nc.scalar.activation(mu[:], S1[:], AF.Copy, scale=1.0 / DIM)
    mu2 = C.tf()
    nc.scalar.activation(mu2[:], mu[:], AF.Square)
    var = C.tf()
    nc.vector.scalar_tensor_tensor(var[:], S2[:], 1.0 / DIM, mu2[:], OP.mult, OP.subtract)
    lv = C.tf()
    nc.scalar.activation(lv[:], var[:], AF.Ln, bias=EPS)
    k = C.tf()
    nc.scalar.activation(k[:], lv[:], AF.Exp, scale=-0.5)
    hs = []
    for i, x in enumerate(xs):
        xm = C.tf()
        nc.vector.tensor_tensor(xm[:], x[:].bitcast(F32), mu[:], OP.subtract)
        h = C.tr()
        nc.gpsimd.tensor_tensor(h[:], xm[:], k[:].bitcast(F32R), OP.mult)
        hs.append(h)

    # feature-major B|C projections
    pb = C.ps((ST, TG))
    for i in range(2):
        nc.tensor.matmul(pb[:], W["WB"][i][:], hs[i][:], start=(i == 0), stop=(i == 1))
    Bd = C.th()
    nc.scalar.activation(Bd[:], pb[:], AF.Copy)
    pc = C.ps((ST, TG))
    for i in range(2):
        nc.tensor.matmul(pc[:], W["WC"][i][:], hs[i][:], start=(i == 0), stop=(i == 1))
    Cd = C.th()
    nc.scalar.activation(Cd[:], pc[:], AF.Copy)

    yfd = C.yf_t()

    for tt in range(NT):
        tsl = slice(tt * 128, (tt + 1) * 128)
        h0s, h1s = hs[0][:, tsl], hs[1][:, tsl]

        pxin = C.ps()
        nc.tensor.matmul(pxin[:], h0s, W["Wx"][0][:], start=True, stop=False)
        nc.tensor.matmul(pxin[:], h1s, W["Wx"][1][:], start=False, stop=True)
        e1 = C.tf()
        nc.scalar.activation(e1[:], pxin[:], AF.Exp, scale=-1.0)
        w1 = C.tf()
        nc.vector.tensor_scalar(w1[:], e1[:], 1.0, None, OP.add)
        r1 = C.tf()
        nc.vector.reciprocal_approx_fast(r1[:], w1[:])
        xsT = C.tb()
        nc.vector.tensor_tensor(xsT[:], pxin[:], r1[:], OP.mult)

        pz = C.ps()
        nc.tensor.matmul(pz[:], h0s, W["Wz"][0][:], start=True, stop=False)
        nc.tensor.matmul(pz[:], h1s, W["Wz"][1][:], start=False, stop=True)
        e2 = C.tf()
        nc.scalar.activation(e2[:], pz[:], AF.Exp, scale=-1.0)
        w2 = C.tf()
        nc.vector.tensor_scalar(w2[:], e2[:], 1.0, None, OP.add)
        r2 = C.tf()
        nc.vector.reciprocal_approx_fast(r2[:], w2[:])
        szT = C.tb()
        nc.vector.tensor_tensor(szT[:], pz[:], r2[:], OP.mult)

        pdt = C.ps()
        nc.tensor.matmul(pdt[:], h0s, W["Wdt"][0][:], start=True, stop=False)
        nc.tensor.matmul(pdt[:], h1s, W["Wdt"][1][:], start=False, stop=False)
        nc.tensor.matmul(pdt[:], C.ones_row[:], W["dt_bias"][:], start=False, stop=True)
        edt = C.tf()
        nc.scalar.activation(edt[:], pdt[:], AF.Exp)
        dtT = C.tf()
        nc.scalar.activation(dtT[:], edt[:], AF.Ln, bias=1.0)

        pbt = C.pss((128, ST))
        nc.tensor.matmul(pbt[:], h0s, W["WB"][0][:], start=True, stop=False)
        nc.tensor.matmul(pbt[:], h1s, W["WB"][1][:], start=False, stop=True)
        BT = C.ts((128, ST), BF16)
        nc.scalar.activation(BT[:], pbt[:], AF.Copy)

        dtA = C.tr()
        nc.vector.tensor_tensor(dtA[:], dtT[:], W["A_bc"][:], OP.mult)
        pa = C.ps()
        ltri = C.ltri_f if fwd else C.ltri_b
        nc.tensor.matmul(pa[:], ltri[:], dtA[:], start=True, stop=True)
        PT = C.tb()
        nc.scalar.activation(PT[:], pa[:], AF.Exp)
        Pi = C.tb()
        nc.scalar.activation(Pi[:], pa[:], AF.Exp, scale=-1.0)
        uT = C.tb()
        nc.vector.tensor_tensor(uT[:], dtT[:], xsT[:], OP.mult)
        vT = C.tb()
        nc.vector.tensor_tensor(vT[:], uT[:], Pi[:], OP.mult)

        c0 = slice(tt * 128, tt * 128 + 64)
        c1 = slice(tt * 128 + 64, tt * 128 + 128)
        psm0 = C.pss((ST, ST))
        nc.tensor.matmul(psm0[:], Bd[:, c0], Cd[:, c0], start=True, stop=True)
        psm1 = C.pss((ST, ST))
        nc.tensor.matmul(psm1[:], Bd[:, c1], Cd[:, c1], start=True, stop=True)
        SMTm = C.ts((128, ST), BF16)
        mask = C.mask_f if fwd else C.mask_b
        nc.vector.tensor_tensor(SMTm[0:64, :], psm0[:], mask[0:64, :], OP.mult)
        nc.vector.tensor_tensor(SMTm[64:128, :], psm1[:], mask[64:128, :], OP.mult)

        py0 = C.ps((ST, DI))
        py1 = C.ps((ST, DI))
        if fwd:
            srcp, csum, cdst, pdst = slice(0, 64), C.csum_f, c1, py1
        else:
            srcp, csum, cdst, pdst = slice(64, 128), C.csum_b, c0, py0
        pu = C.ps((ST, DI))
        nc.tensor.matmul(pu[:], BT[srcp, :], vT[srcp, :], start=True, stop=True)
        pw = C.ps((ST, DI))
        nc.tensor.matmul(pw[:], csum[:], dtA[:], start=True, stop=True)
        wend = C.th()
        nc.scalar.activation(wend[:], pw[:], AF.Exp)
        Hst = C.th()
        nc.vector.tensor_tensor(Hst[:], pu[:], wend[:].bitcast(F32), OP.mult)
        nc.tensor.matmul(py0[:], SMTm[0:64, :], vT[0:64, :], start=True, stop=fwd)
        nc.tensor.matmul(py1[:], SMTm[64:128, :], vT[64:128, :], start=True, stop=not fwd)
        nc.tensor.matmul(pdst[:], Cd[:, cdst], Hst[:], start=False, stop=True)

        q1 = C.tb()
        nc.vector.tensor_tensor(q1[0:64, :], py0[:], PT[0:64, :], OP.mult)
        nc.vector.tensor_tensor(q1[64:128, :], py1[:], PT[64:128, :], OP.mult)
        q3 = C.tb()
        nc.vector.tensor_tensor(q3[:], q1[:], xsT[:], OP.add)
        yf = C.tb()
        nc.vector.tensor_tensor(yf[:], q3[:], szT[:], OP.mult)

        # 4 transposed 128x128 blocks land in ONE psum bank; one act drains it
        ptr = C.pss((128, TG), BF16)
        for db in range(4):
            nc.tensor.transpose(ptr[:, db * 128:(db + 1) * 128],
                                yf[:, db * 128:(db + 1) * 128], C.identB[:])
        dst = yfd[:].rearrange("p (db t) -> p db t", db=4)[:, :, tt * 128:(tt + 1) * 128]
        src = ptr[:].rearrange("p (db t) -> p db t", db=4)
        nc.scalar.activation(dst, src, AF.Copy)

    nxs = []
    for mo in range(2):
        po = C.ps()
        for db in range(4):
            nc.tensor.matmul(po[:], W["Wout"][db][:, mo * 128:(mo + 1) * 128],
                             yfd[:, db * TG:(db + 1) * TG],
                             start=(db == 0), stop=False)
        nc.tensor.matmul(po[:], C.identR[:], xs[mo][:], start=False, stop=True)
        nx = C.xt()
        nc.scalar.activation(nx[:], po[:], AF.Copy)
        nxs.append(nx)
    return nxs


def emit_load_group(C, D, g):
    nc = C.nc
    xs = [C.xt() for _ in range(2)]
    for tt in range(NT):
        xT = C.ts((128, DIM))
        nc.sync.dma_start(xT[:], D["dtok"][(4 * g + tt) * 128:(4 * g + tt + 1) * 128, :])
        for i in range(2):
            ptr = C.pss((128, 128))
            nc.tensor.transpose(ptr[:], xT[:, i * 128:(i + 1) * 128], C.ident[:])
            nc.scalar.activation(xs[i][:, tt * 128:(tt + 1) * 128], ptr[:], AF.Copy)
    return xs


def emit_gate(C, fts, bts, gW):
    nc = C.nc
    merged = []
    for mo in range(2):
        pg = C.ps()
        ins = [fts[0], fts[1], bts[0], bts[1]]
        for kb in range(4):
            nc.tensor.matmul(pg[:], gW[kb][:, mo * 128:(mo + 1) * 128], ins[kb][:],
                             start=(kb == 0), stop=(kb == 3))
        ge = C.tf()
        nc.scalar.activation(ge[:], pg[:], AF.Exp, scale=-1.0)
        gsp = C.tf()
        nc.scalar.activation(gsp[:], ge[:], AF.Ln, bias=1.0)
        gate = C.tf()
        nc.scalar.activation(gate[:], gsp[:], AF.Exp, scale=-1.0)
        d = C.tf()
        nc.vector.tensor_tensor(d[:], fts[mo][:].bitcast(F32), bts[mo][:].bitcast(F32),
                                OP.subtract)
        m1 = C.tf()
        nc.vector.tensor_tensor(m1[:], gate[:], d[:], OP.mult)
        mg = C.mt()
        nc.vector.tensor_tensor(mg[:], m1[:], bts[mo][:].bitcast(F32), OP.add)
        merged.append(mg)
    return merged


_TABLES_PATCHED = False


def _pin_act_table():
    # Force every ACT instruction onto natural_log_exp_and_others (covers our
    # Exp/Ln/Square/Copy/Identity) so bacc never inserts per-function table
    # reloads (~1.3us each).
    global _TABLES_PATCHED
    if _TABLES_PATCHED:
        return
    import concourse.bacc as _bacc
    _orig = _bacc.get_activation_tables

    def _pinned(arch):
        t = _orig(arch)
        return {k: (v if k == "natural_log_exp_and_others" else set())
                for k, v in t.items()}

    _bacc.get_activation_tables = _pinned
    _TABLES_PATCHED = True


def build_nc():
    """Build the full SPMD program (uniform across cores)."""
    _pin_act_table()
    nc = bacc.Bacc(trn_type="TRN2", target_bir_lowering=False, debug=False,
                   enable_asserts=False, num_devices=8)
    epst = nc.alloc_sbuf_tensor("const-eps", [128, 1], F32)
    nc.gpsimd.memset(epst.ap(), EPS)
    nc.const_aps.aps[(F32, EPS)] = epst.ap()
    nc.all_engine_barrier()
    D = {}

    def inp(name, shape, dtype=F32R):
        D[name] = nc.dram_tensor(name, list(shape), dtype, kind="ExternalInput").ap()

    inp("dtok", (NTOK, DIM), F32)
    inp("Wx", (2, NL, DIM, DI)); inp("Wz", (2, NL, DIM, DI)); inp("Wdt", (2, NL, DIM, DI))
    inp("WB", (2, NL, DIM, ST)); inp("WC", (2, NL, DIM, ST))
    inp("Wout", (2, NL, DI, DIM))
    inp("dt_bias", (2, NL, 1, DI))
    inp("A_bc", (2, NL, 128, DI), F32)
    inp("gate_W", (DI, DIM))
    inp("merge_W", (2, DI, DIM), BF16)
    inp("ones128", (128, 128)); inp("ones_row", (1, 128))
    inp("ident", (128, 128), F32); inp("identR", (128, 128))
    inp("identB", (128, 128), BF16)
    inp("ltri_f", (128, 128)); inp("ltri_b", (128, 128))
    inp("mask_f", (128, ST), F32); inp("mask_b", (128, ST), F32)
    inp("csum_f", (128, ST)); inp("csum_b", (128, ST))
    inp("vmask", (128, 1), F32)

    osem = nc.dram_tensor("osem", [SH * SW, DIM], F32, kind="ExternalOutput").ap()
    oinst = nc.dram_tensor("oinst", [SH * SW, DIM], F32, kind="ExternalOutput").ap()
    fsp = nc.dram_tensor("fsp", [16, 128, TG], F32R).ap()

    with tile.TileContext(nc) as tc, ExitStack() as ctx:
        C = KC(nc, tc, ctx)
        cp = C.pool("consts", 1)

        def cload(name, shape, dtype=F32R):
            t = cp.tile(list(shape), dtype, tag=name, name=name)
            nc.sync.dma_start(t[:], D[name][:])
            return t

        C.ones128 = cload("ones128", (128, 128))
        C.ones_row = cload("ones_row", (1, 128))
        C.ident = cload("ident", (128, 128), F32)
        C.identB = cload("identB", (128, 128), BF16)
        C.identR = cload("identR", (128, 128))
        C.ltri_f = cload("ltri_f", (128, 128))
        C.ltri_b = cload("ltri_b", (128, 128))
        C.mask_f = cload("mask_f", (128, ST), F32)
        C.mask_b = cload("mask_b", (128, ST), F32)
        C.csum_f = cload("csum_f", (128, ST))
        C.csum_b = cload("csum_b", (128, ST))
        C.vmask = cload("vmask", (128, 1), F32)

        _build_full(C, D, osem, oinst, fsp)

    nc.compile()
    return nc


def _load_gate_w(C, D):
    nc = C.nc
    gW = []
    for kb in range(4):
        t = C.wt(f"gW{kb}", [128, DIM])
        nc.sync.dma_start(t[:], D["gate_W"][kb * 128:(kb + 1) * 128, :])
        gW.append(t)
    return gW


def _build_full(C, D, osem, oinst, fsp):
    nc = C.nc

    # self results (this core's direction), feature-major, group-major tokens
    Rt = [C.pool("Rt", 1).tile([128, 8 * TG], BF16, tag=f"Rt{k}", name=f"Rt{k}")
          for k in range(2)]
    # masked copy in ROW-MAJOR token order assuming v-scan source: real
    # permutation on v-cores, scrambled zeros on (masked) h-cores
    Rv = [C.pool("Rv", 1).tile([128, 8 * TG], BF16, tag=f"Rv{k}", name=f"Rv{k}")
          for k in range(2)]
    gW = _load_gate_w(C, D)

    dramp = C.pool("ccd", 1, space="DRAM")
    cc_in = [dramp.tile([256, 2 * TG], BF16, tag=f"cc_in{h}", name=f"cc_in{h}")
             for h in range(4)]
    cc_out = [dramp.tile([256, 2 * TG], BF16, tag=f"cc_out{h}", name=f"cc_out{h}")
              for h in range(4)]

    # ---------------- single-direction bicms ----------------
    for half in range(2):
        g0 = half * 4
        x_groups = [emit_load_group(C, D, g0 + g) for g in range(4)]
        for l in range(NL):
            W = emit_load_weights(C, D, 0, l)
            x_groups = [emit_layer(C, x_groups[g], W, True) for g in range(4)]
        for g in range(4):
            for i, t in enumerate(x_groups[g]):
                nc.sync.dma_start(fsp[2 * (g0 + g) + i, :, :], t[:])
        x_groups = [emit_load_group(C, D, g0 + g) for g in range(4)]
        W = emit_load_weights(C, D, 1, 0)
        x_groups = [emit_layer(C, x_groups[g], W, False) for g in range(4)]
        W = emit_load_weights(C, D, 1, 1)
        for g in range(4):
            gg = g0 + g
            bo = emit_layer(C, x_groups[g], W, False)
            f0 = C.tr(); nc.sync.dma_start(f0[:], fsp[2 * gg + 0, :, :])
            f1 = C.tr(); nc.sync.dma_start(f1[:], fsp[2 * gg + 1, :, :])
            mg = emit_gate(C, [f0, f1], bo, gW)
            for k in range(2):
                nc.scalar.activation(Rt[k][:, gg * TG:(gg + 1) * TG],
                                     mg[k][:].bitcast(F32), AF.Copy)
                # v-scan group gg covers cols w in [8*gg, 8*gg+8); scatter to
                # row-major flat (row*SW + w)*2 + s, scaled by the v-mask
                dstv = Rv[k][:].rearrange("p (h w two) -> p w h two",
                                          h=SH, w=SW, two=2)[:, 8 * gg:8 * gg + 8]
                srcv = mg[k][:].rearrange("p (wl h two) -> p wl h two",
                                          wl=8, h=SH, two=2)
                nc.scalar.activation(dstv, srcv, AF.Copy, scale=C.vmask[:])
            if g % 2 == 1:
                # exchange the finished 16-column quarter while compute continues
                qq = gg // 2
                for k in range(2):
                    csrc = Rv[k][:].rearrange(
                        "p (h w two) -> p h w two",
                        h=SH, w=SW, two=2)[:, :, 16 * qq:16 * qq + 16]
                    nc.sync.dma_start(
                        cc_in[qq][k * 128:(k + 1) * 128, :].rearrange(
                            "p (h w two) -> p h w two", h=SH, w=16, two=2),
                        csrc)
                nc.gpsimd.collective_compute(
                    "AllReduce", OP.add,
                    replica_groups=[[0, 1], [2, 3], [4, 5], [6, 7]],
                    ins=[cc_in[qq].opt()],
                    outs=[cc_out[qq].opt()],
                )



    # ---------------- merge: self-as-H + peer-as-V -------------------------
    Pv = [C.pool("Pv", 1).tile([128, 8 * TG], BF16, tag=f"Pv{k}", name=f"Pv{k}")
          for k in range(2)]
    for k in range(2):
        for h in range(4):
            dstp = Pv[k][:].rearrange("p (hh w two) -> p hh w two",
                                      hh=SH, w=SW, two=2)[:, :, 16 * h:16 * h + 16]
            nc.sync.dma_start(
                dstp,
                cc_out[h][k * 128:(k + 1) * 128, :].rearrange(
                    "p (hh w two) -> p hh w two", hh=SH, w=16, two=2))
    mW = {}
    for s in range(2):
        mW[s] = []
        for kb in range(4):
            t = C.wt(f"mW{s}{kb}", [128, DIM], BF16)
            nc.sync.dma_start(t[:], D["merge_W"][s, kb * 128:(kb + 1) * 128, :])
            mW[s].append(t)

    outd = {0: osem, 1: oinst}
    for gg in range(8):
        _emit_merge_tiles(C, Rt, Pv, gg, mW, outd)


def _emit_merge_tiles(C, Rt, Pv, g, mW, outd):
    nc = C.nc
    statg = C.pool("stat", 2).tile([128, 16], F32, tag="statg", name="statg")
    pms = {}
    for s in range(2):
        for r in range(2):
            i = 2 * g + r
            pm = C.ps((128, DIM))
            for k in range(2):
                srcp = Rt[k][:, g * TG:(g + 1) * TG].rearrange(
                    "p (rl w two) -> p rl w two",
                    rl=4, w=SW, two=2)[:, 2 * r:2 * r + 2, :, s]
                nc.tensor.matmul(pm[:], srcp, mW[s][k][:], start=(k == 0), stop=False)
            for k in range(2):
                srcv = Pv[k][:, i * 256:(i + 1) * 256].rearrange(
                    "p (t two) -> p t two", t=128, two=2)[:, :, s]
                nc.tensor.matmul(pm[:], srcv, mW[s][2 + k][:], start=False, stop=(k == 1))

            sc1 = C.ts((128, DIM))
            ci = (s * 2 + r) * 2
            nc.scalar.activation(sc1[:], pm[:], AF.Copy,
                                 accum_out=statg[:, ci:ci + 1])
            sc2 = C.ts((128, DIM))
            nc.scalar.activation(sc2[:], pm[:], AF.Square,
                                 accum_out=statg[:, ci + 1:ci + 2])
            pms[(s, r)] = sc1
    mu = C.pool("stat", 2).tile([128, 4], F32, tag="mu", name="mu")
    nc.scalar.activation(mu[:], statg[:, 0:8:2], AF.Copy, scale=1.0 / DIM)
    mu2 = C.pool("stat", 2).tile([128, 4], F32, tag="mu2", name="mu2")
    nc.scalar.activation(mu2[:], mu[:], AF.Square)
    var = C.pool("stat", 2).tile([128, 4], F32, tag="var", name="var")
    nc.vector.scalar_tensor_tensor(var[:], statg[:, 1:8:2], 1.0 / DIM, mu2[:],
                                   OP.mult, OP.subtract)
    sdt = C.pool("stat", 2).tile([128, 4], F32, tag="sdt", name="sdt")
    nc.scalar.activation(sdt[:], var[:], AF.Ln, bias=EPS)
    rs = C.pool("stat", 2).tile([128, 4], F32, tag="rs", name="rs")
    nc.scalar.activation(rs[:], sdt[:], AF.Exp, scale=-0.5)
    for s in range(2):
        for r in range(2):
            i = 2 * g + r
            ci = s * 2 + r
            sc1 = pms[(s, r)]
            t1 = C.ts((128, DIM))
            nc.vector.tensor_scalar(t1[:], sc1[:], mu[:, ci:ci + 1],
                                    rs[:, ci:ci + 1], OP.subtract, OP.mult)
            nc.sync.dma_start(outd[s][i * 128:(i + 1) * 128, :], t1[:])


# ---------------------------------------------------------------------------
# host side
# ---------------------------------------------------------------------------

_CACHE = {}


def _consts_np():
    import ml_dtypes
    q = 64
    tri = np.tril(np.ones((q, q), np.float32))          # tri[t, tau] t>=tau
    ltri_f = np.zeros((128, 128), np.float32)           # [tau, t] = tau<=t
    ltri_b = np.zeros((128, 128), np.float32)           # [tau, t] = tau>=t
    for c in range(2):
        ltri_f[c * q:(c + 1) * q, c * q:(c + 1) * q] = tri.T
        ltri_b[c * q:(c + 1) * q, c * q:(c + 1) * q] = tri
    mask_f = np.zeros((128, q), np.float32)
    mask_b = np.zeros((128, q), np.float32)
    for c in range(2):
        mask_f[c * q:(c + 1) * q, :] = tri.T
        mask_b[c * q:(c + 1) * q, :] = tri
    return {
        "ones128": np.ones((128, 128), np.float32),
        "ones_row": np.ones((1, 128), np.float32),
        "ident": np.eye(128, dtype=np.float32),
        "identB": np.eye(128).astype(ml_dtypes.bfloat16),
        "identR": np.eye(128, dtype=np.float32),
        "ltri_f": ltri_f, "ltri_b": ltri_b,
        "mask_f": mask_f, "mask_b": mask_b,
        "csum_f": np.concatenate([np.ones((64, 64), np.float32),
                                  np.zeros((64, 64), np.float32)]),
        "csum_b": np.concatenate([np.zeros((64, 64), np.float32),
                                  np.ones((64, 64), np.float32)]),
    }


def _check_trivial_params(inputs):
    """The compiled program folds away ops for parameters that are
    structurally trivial in this problem's setup_inputs (ln affine = identity,
    Dskip = 1, zero biases on gate/merge, identity merge LN affine). Verify
    that assumption on the actual inputs rather than trusting it silently."""
    f = lambda x: np.asarray(x, np.float32)
    checks = [
        (np.all(f(inputs["ln_s"]) == 1.0), "ln_s != 1"),
        (np.all(f(inputs["ln_b"]) == 0.0), "ln_b != 0"),
        (np.all(f(inputs["Dskip"]) == 1.0), "Dskip != 1"),
        (np.all(f(inputs["gate_b"]) == 0.0), "gate_b != 0"),
        (np.all(f(inputs["merge_b"]) == 0.0), "merge_b != 0"),
        (np.all(f(inputs["merge_ln_s"]) == 1.0), "merge_ln_s != 1"),
        (np.all(f(inputs["merge_ln_b"]) == 0.0), "merge_ln_b != 0"),
    ]
    bad = [msg for ok, msg in checks if not ok]
    if bad:
        raise NotImplementedError(
            f"kernel compiled for trivial affine/bias params; got {bad}")


def prep_inputs(inputs):
    """Per-core input maps: core 2b = horizontal bicms of image b, core
    2b+1 = vertical. Direction is encoded entirely in the data/weights."""
    import ml_dtypes
    f = lambda x: np.ascontiguousarray(np.asarray(x, np.float32))
    consts = _consts_np()
    Wfull = {n: f(inputs[n]) for n in ("Wx", "Wz", "Wdt", "WB", "WC", "Wout")}
    dt_bias = f(inputs["dt_bias"]).reshape(4, NL, 1, DI)
    A = -np.exp(f(inputs["A_log"]))
    A_bc = np.ascontiguousarray(np.broadcast_to(A[:, :, None, :], (4, NL, 128, DI)))
    gate_W = f(inputs["gate_W"])
    merge_W = f(inputs["merge_W"]).astype(ml_dtypes.bfloat16)
    csum_kill = np.full((128, ST), 1.0e4, np.float32)

    sem4 = f(inputs["stream_sem"]).reshape(BATCH, SH, SW, DIM)
    inst4 = f(inputs["stream_inst"]).reshape(BATCH, SH, SW, DIM)
    in_maps = []
    for c in range(8):
        b, p = c // 2, c % 2
        m = dict(consts)
        if p == 0:   # horizontal: row-major scan
            s2 = sem4[b].reshape(SH * SW, DIM)
            i2 = inst4[b].reshape(SH * SW, DIM)
        else:        # vertical: column-major scan
            s2 = np.ascontiguousarray(sem4[b].transpose(1, 0, 2)).reshape(SH * SW, DIM)
            i2 = np.ascontiguousarray(inst4[b].transpose(1, 0, 2)).reshape(SH * SW, DIM)
        m["dtok"] = np.ascontiguousarray(
            np.stack([s2, i2], axis=1).reshape(NTOK, DIM))
        sl = slice(2 * p, 2 * p + 2)   # h: stacks 0,1;  v: stacks 2,3
        for n in ("Wx", "Wz", "Wdt", "WB", "WC", "Wout"):
            m[n] = Wfull[n][sl]
        m["dt_bias"] = dt_bias[sl]
        m["A_bc"] = A_bc[sl]
        m["gate_W"] = gate_W[p]
        m["merge_W"] = merge_W
        if p == 1:
            m["csum_f"] = csum_kill
            m["csum_b"] = csum_kill
        m["vmask"] = np.full((128, 1), float(p), np.float32)
        in_maps.append(m)
    return in_maps


def kernel(**inputs):
    from concourse.bass_utils import run_bass_kernel_spmd
    _check_trivial_params(inputs)
    if "nc" not in _CACHE:
        _CACHE["nc"] = build_nc()
    nc = _CACHE["nc"]
    in_maps = prep_inputs(inputs)
    res = run_bass_kernel_spmd(nc, in_maps, list(range(8)))
    fused_sem = np.stack([res.results[2 * b]["osem"] for b in range(BATCH)])
    fused_inst = np.stack([res.results[2 * b]["oinst"] for b in range(BATCH)])
    return fused_sem, fused_inst


def timed_run(inputs, iters=6):
    """Steady-state device execution time per kernel invocation.

    A single synchronous dispatch through the axon/PJRT tunnel carries a
    ~74 ms fixed host<->terminal round trip (measured: a one-DMA trivial
    kernel times 73.7 ms), which swamps the actual on-device time. To
    measure hardware execution, enqueue K back-to-back executions
    asynchronously (PJRT serializes them per core) and take the marginal
    cost d(total)/dK between two batch sizes: the fixed RTT and dispatch
    pipeline cancel, leaving per-execution device time.
    """
    import time
    import jax
    import numpy as np_
    from jax.sharding import Mesh, PartitionSpec, NamedSharding
    from jax.experimental.shard_map import shard_map
    import concourse.mybir as mybir_
    from concourse import bass2jax

    if "nc" not in _CACHE:
        _CACHE["nc"] = build_nc()
    nc = _CACHE["nc"]
    in_maps = prep_inputs(inputs)
    n_cores = 8

    bass2jax.install_neuronx_cc_hook()
    partition_name = nc.partition_id_tensor.name if nc.partition_id_tensor else None
    in_names, out_names, out_avals, zero_outs = [], [], [], []
    for alloc in nc.m.functions[0].allocations:
        if not isinstance(alloc, mybir_.MemoryLocationSet):
            continue
        name = alloc.memorylocations[0].name
        if alloc.kind == "ExternalInput":
            if name != partition_name:
                in_names.append(name)
        elif alloc.kind == "ExternalOutput":
            shape = tuple(alloc.tensor_shape)
            dtype = mybir_.dt.np(alloc.dtype)
            out_names.append(name)
            out_avals.append(jax.core.ShapedArray(shape, dtype))
            zero_outs.append(np_.zeros(shape, dtype))
    n_params = len(in_names)
    n_outs = len(out_avals)
    all_in_names = list(in_names) + list(out_names)
    if partition_name is not None:
        all_in_names.append(partition_name)

    def _body(*args):
        operands = list(args)
        if partition_name is not None:
            operands.append(bass2jax.partition_id_tensor())
        return tuple(bass2jax._bass_exec_p.bind(
            *operands, out_avals=tuple(out_avals), in_names=tuple(all_in_names),
            out_names=tuple(out_names), lowering_input_output_aliases=(),
            sim_require_finite=True, sim_require_nnan=True, nc=nc))

    devices = jax.devices()[:n_cores]
    mesh = Mesh(np_.asarray(devices), ("core",))
    spec = PartitionSpec("core")
    fn = jax.jit(
        shard_map(_body, mesh=mesh, in_specs=(spec,) * (n_params + n_outs),
                  out_specs=(spec,) * n_outs, check_rep=False),
        keep_unused=True)
    sh = NamedSharding(mesh, spec)
    dev_in = [jax.device_put(
        np_.concatenate([np_.asarray(in_maps[c][nm]) for c in range(n_cores)], 0), sh)
        for nm in in_names]
    zs = [jax.device_put(np_.concatenate([z] * n_cores, 0), sh) for z in zero_outs]
    outs = fn(*dev_in, *zs)  # warmup (compile + first dispatch)
    jax.block_until_ready(outs)

    def batch_time(k):
        best = None
        for _ in range(max(2, iters // 2)):
            t0 = time.perf_counter()
            outs_list = [fn(*dev_in, *zs) for _ in range(k)]
            jax.block_until_ready(outs_list)
            dt = time.perf_counter() - t0
            best = dt if best is None else min(best, dt)
        return best

    k1, k2 = 4, 24
    t1, t2 = batch_time(k1), batch_time(k2)
    return int((t2 - t1) / (k2 - k1) * 1e9)


# revision 45
# speedup vs baseline: 1.2541x; 1.2541x over previous
"""Trainium2 Bass kernel for nn_FourDirectionalCrossModalScan.

SPMD over 8 NeuronCores, one batch element per core PAIR: core 2b runs the
HORIZONTAL bicms of image b, core 2b+1 the VERTICAL bicms. The program is
identical on every core — direction is selected purely by inputs:
  - dtok: tokens pre-interleaved (sem,inst) in this core's scan order
  - weight slots 0/1 hold this core's (fwd, bwd) stacks (host-reordered)
  - csum_f/csum_b: real cumsum-select for h-cores; +1e4 on v-cores, which
    drives the chunk-carry weight exp(csum@dtA) to 0 (dtA<0 strictly), so
    the two 64-token chunks of a tile stay independent sequences
After the gated bicms, each core's result tiles are written (scaled by a
per-core {0,1} mask) to a DRAM bounce buffer; a pairwise AllReduce(add)
leaves the v-core's results on both cores. Every core then merges
self-as-H with peer-as-V (a compile-time col-major->row-major access
pattern) and LayerNorms — correct on even cores, garbage-but-finite on odd
cores, and the host reads even cores only.

The Mamba scan uses a chunked (SSD) decomposition with chunk Q=64:
  P(t,d) = exp(cumsum_chunk(dt*A))   -- token-major via block-tri matmul + Exp
  v      = dt*silu(xin) / P
  SMT    = B_c^T C_c (per chunk, causal-masked)
  y(t,d) = P * [ SMT^T v  +  C^T H_prev ],   H = P_end * (B^T v)

Perf structure: the exponent path (dt -> dt*A -> cumsum -> exp) stays fp32
for precision; value-path tensors (xs, sz, u, v, P, 1/P, B^T, SMT, q*, yf)
are bf16 (2x-rate DVE, full-rate PE). The four yf transposes land in one
PSUM bank drained by one activation. Ops that are identities for this
problem's parameters (ln affine, Dskip, gate/merge biases, merge-LN
affine) are compiled out; kernel() verifies those assumptions on the
actual inputs and refuses to run on violation.
"""
import numpy as np
from contextlib import ExitStack

import concourse.bass as bass
import concourse.bacc as bacc
import concourse.tile as tile
import concourse.mybir as mybir

F32 = mybir.dt.float32
F32R = mybir.dt.float32r
BF16 = mybir.dt.bfloat16
AF = mybir.ActivationFunctionType
OP = mybir.AluOpType

DIM = 256
DI = 512
ST = 64
NL = 2
SH, SW = 32, 64
BATCH = 4
EPS = 1e-5
TG = 512
NT = 4
NTOK = 2 * SH * SW  # 4096 interleaved tokens per direction


class KC:
    def __init__(self, nc, tc, ctx):
        self.nc = nc
        self.tc = tc
        self.ctx = ctx
        self.pools = {}

    def pool(self, name, bufs, space="SBUF"):
        if name not in self.pools:
            self.pools[name] = self.ctx.enter_context(
                self.tc.tile_pool(name=name, bufs=bufs, space=space))
        return self.pools[name]

    def tf(self, shape=(128, TG)):
        return self.pool("tf", 7).tile(list(shape), F32, tag="tf", name="tf")

    def tb(self, shape=(128, TG)):
        return self.pool("tb", 12).tile(list(shape), BF16, tag="tb", name="tb")

    def tr(self, shape=(128, TG)):
        return self.pool("tr", 10).tile(list(shape), F32R, tag="tr", name="tr")

    def ts(self, shape, dtype=F32):
        return self.pool("ts", 9).tile(list(shape), dtype, tag="ts", name="ts")

    def th(self):
        return self.pool("th", 5).tile([64, DI], F32R, tag="th", name="th")

    def ps(self, shape=(128, TG), dtype=F32):
        return self.pool("psA", 6, space="PSUM").tile(list(shape), dtype, tag="ps", name="ps")

    def pss(self, shape=(128, 128), dtype=F32):
        return self.pool("psS", 2, space="PSUM").tile(list(shape), dtype, tag="pss", name="pss")

    def wt(self, tag, shape, dtype=F32R, big=False):
        return self.pool("wbig" if big else "wrest", 2 if big else 1).tile(
            list(shape), dtype, tag=tag, name=tag)

    def xt(self):
        return self.pool("xio", 10).tile([128, TG], F32R, tag="xc", name="xc")

    def yf_t(self):
        return self.pool("yfd", 2).tile([128, 4 * TG], F32R, tag="yfd", name="yfd")

    def mt(self):
        return self.pool("mio", 4).tile([128, TG], F32R, tag="mg", name="mg")


def emit_load_weights(C, D, si, l):
    nc = C.nc
    W = {}
    for name in ("Wx", "Wz", "Wdt"):
        W[name] = []
        for i in range(2):
            t = C.wt(f"{name}{i}", [128, DI], big=True)
            nc.sync.dma_start(t[:], D[name][si, l, i * 128:(i + 1) * 128, :])
            W[name].append(t)
    for name in ("WB", "WC"):
        W[name] = []
        for i in range(2):
            t = C.wt(f"{name}{i}", [128, ST])
            nc.sync.dma_start(t[:], D[name][si, l, i * 128:(i + 1) * 128, :])
            W[name].append(t)
    W["Wout"] = []
    for db in range(4):
        t = C.wt(f"Wout{db}", [128, DIM])
        nc.sync.dma_start(t[:], D["Wout"][si, l, db * 128:(db + 1) * 128, :])
        W["Wout"].append(t)
    W["dt_bias"] = C.wt("dtb", [1, DI])
    nc.sync.dma_start(W["dt_bias"][:], D["dt_bias"][si, l])
    t = C.wt("A_bc", [128, DI], F32)
    nc.sync.dma_start(t[:], D["A_bc"][si, l])
    W["A_bc"] = t
    return W


def emit_layer(C, xs, W, fwd):
    nc = C.nc

    # layernorm stats (feature-major; cross-partition sums via ones-matmul)
    S1 = C.ps()
    for i, x in enumerate(xs):
        nc.tensor.matmul(S1[:], C.ones128[:], x[:], start=(i == 0), stop=(i == 1))
    S2 = C.ps()
    for i, x in enumerate(xs):
        sq = C.tr()
        nc.scalar.activation(sq[:], x[:].bitcast(F32), AF.Square)
        nc.tensor.matmul(S2[:], C.ones128[:], sq[:], start=(i == 0), stop=(i == 1))
    mu = C.tf()
    nc.scalar.activation(mu[:], S1[:], AF.Copy, scale=1.0 / DIM)
    mu2 = C.tf()
    nc.scalar.activation(mu2[:], mu[:], AF.Square)
    var = C.tf()
    nc.vector.scalar_tensor_tensor(var[:], S2[:], 1.0 / DIM, mu2[:], OP.mult, OP.subtract)
    lv = C.tf()
    nc.scalar.activation(lv[:], var[:], AF.Ln, bias=EPS)
    k = C.tf()
    nc.scalar.activation(k[:], lv[:], AF.Exp, scale=-0.5)
    hs = []
    for i, x in enumerate(xs):
        xm = C.tf()
        nc.vector.tensor_tensor(xm[:], x[:].bitcast(F32), mu[:], OP.subtract)
        h = C.tr()
        nc.gpsimd.tensor_tensor(h[:], xm[:], k[:].bitcast(F32R), OP.mult)
        hs.append(h)

    # feature-major B|C projections
    pb = C.ps((ST, TG))
    for i in range(2):
        nc.tensor.matmul(pb[:], W["WB"][i][:], hs[i][:], start=(i == 0), stop=(i == 1))
    Bd = C.th()
    nc.scalar.activation(Bd[:], pb[:], AF.Copy)
    pc = C.ps((ST, TG))
    for i in range(2):
        nc.tensor.matmul(pc[:], W["WC"][i][:], hs[i][:], start=(i == 0), stop=(i == 1))
    Cd = C.th()
    nc.scalar.activation(Cd[:], pc[:], AF.Copy)

    yfd = C.yf_t()

    for tt in range(NT):
        tsl = slice(tt * 128, (tt + 1) * 128)
        h0s, h1s = hs[0][:, tsl], hs[1][:, tsl]

        pxin = C.ps()
        nc.tensor.matmul(pxin[:], h0s, W["Wx"][0][:], start=True, stop=False)
        nc.tensor.matmul(pxin[:], h1s, W["Wx"][1][:], start=False, stop=True)
        e1 = C.tf()
        nc.scalar.activation(e1[:], pxin[:], AF.Exp, scale=-1.0)
        w1 = C.tf()
        nc.vector.tensor_scalar(w1[:], e1[:], 1.0, None, OP.add)
        r1 = C.tf()
        nc.vector.reciprocal_approx_fast(r1[:], w1[:])
        xsT = C.tb()
        nc.vector.tensor_tensor(xsT[:], pxin[:], r1[:], OP.mult)

        pz = C.ps()
        nc.tensor.matmul(pz[:], h0s, W["Wz"][0][:], start=True, stop=False)
        nc.tensor.matmul(pz[:], h1s, W["Wz"][1][:], start=False, stop=True)
        e2 = C.tf()
        nc.scalar.activation(e2[:], pz[:], AF.Exp, scale=-1.0)
        w2 = C.tf()
        nc.vector.tensor_scalar(w2[:], e2[:], 1.0, None, OP.add)
        r2 = C.tf()
        nc.vector.reciprocal_approx_fast(r2[:], w2[:])
        szT = C.tb()
        nc.vector.tensor_tensor(szT[:], pz[:], r2[:], OP.mult)

        pdt = C.ps()
        nc.tensor.matmul(pdt[:], h0s, W["Wdt"][0][:], start=True, stop=False)
        nc.tensor.matmul(pdt[:], h1s, W["Wdt"][1][:], start=False, stop=False)
        nc.tensor.matmul(pdt[:], C.ones_row[:], W["dt_bias"][:], start=False, stop=True)
        edt = C.tf()
        nc.scalar.activation(edt[:], pdt[:], AF.Exp)
        dtT = C.tf()
        nc.scalar.activation(dtT[:], edt[:], AF.Ln, bias=1.0)

        pbt = C.pss((128, ST))
        nc.tensor.matmul(pbt[:], h0s, W["WB"][0][:], start=True, stop=False)
        nc.tensor.matmul(pbt[:], h1s, W["WB"][1][:], start=False, stop=True)
        BT = C.ts((128, ST), BF16)
        nc.scalar.activation(BT[:], pbt[:], AF.Copy)

        dtA = C.tr()
        nc.vector.tensor_tensor(dtA[:], dtT[:], W["A_bc"][:], OP.mult)
        pa = C.ps()
        ltri = C.ltri_f if fwd else C.ltri_b
        nc.tensor.matmul(pa[:], ltri[:], dtA[:], start=True, stop=True)
        PT = C.tb()
        nc.scalar.activation(PT[:], pa[:], AF.Exp)
        Pi = C.tb()
        nc.scalar.activation(Pi[:], pa[:], AF.Exp, scale=-1.0)
        uT = C.tb()
        nc.vector.tensor_tensor(uT[:], dtT[:], xsT[:], OP.mult)
        vT = C.tb()
        nc.vector.tensor_tensor(vT[:], uT[:], Pi[:], OP.mult)

        c0 = slice(tt * 128, tt * 128 + 64)
        c1 = slice(tt * 128 + 64, tt * 128 + 128)
        psm0 = C.pss((ST, ST))
        nc.tensor.matmul(psm0[:], Bd[:, c0], Cd[:, c0], start=True, stop=True)
        psm1 = C.pss((ST, ST))
        nc.tensor.matmul(psm1[:], Bd[:, c1], Cd[:, c1], start=True, stop=True)
        SMTm = C.ts((128, ST), BF16)
        mask = C.mask_f if fwd else C.mask_b
        nc.vector.tensor_tensor(SMTm[0:64, :], psm0[:], mask[0:64, :], OP.mult)
        nc.vector.tensor_tensor(SMTm[64:128, :], psm1[:], mask[64:128, :], OP.mult)

        py0 = C.ps((ST, DI))
        py1 = C.ps((ST, DI))
        if fwd:
            srcp, csum, cdst, pdst = slice(0, 64), C.csum_f, c1, py1
        else:
            srcp, csum, cdst, pdst = slice(64, 128), C.csum_b, c0, py0
        pu = C.ps((ST, DI))
        nc.tensor.matmul(pu[:], BT[srcp, :], vT[srcp, :], start=True, stop=True)
        pw = C.ps((ST, DI))
        nc.tensor.matmul(pw[:], csum[:], dtA[:], start=True, stop=True)
        wend = C.th()
        nc.scalar.activation(wend[:], pw[:], AF.Exp)
        Hst = C.th()
        nc.vector.tensor_tensor(Hst[:], pu[:], wend[:].bitcast(F32), OP.mult)
        nc.tensor.matmul(py0[:], SMTm[0:64, :], vT[0:64, :], start=True, stop=fwd)
        nc.tensor.matmul(py1[:], SMTm[64:128, :], vT[64:128, :], start=True, stop=not fwd)
        nc.tensor.matmul(pdst[:], Cd[:, cdst], Hst[:], start=False, stop=True)

        q1 = C.tb()
        nc.vector.tensor_tensor(q1[0:64, :], py0[:], PT[0:64, :], OP.mult)
        nc.vector.tensor_tensor(q1[64:128, :], py1[:], PT[64:128, :], OP.mult)
        q3 = C.tb()
        nc.vector.tensor_tensor(q3[:], q1[:], xsT[:], OP.add)
        yf = C.tb()
        nc.vector.tensor_tensor(yf[:], q3[:], szT[:], OP.mult)

        # 4 transposed 128x128 blocks land in ONE psum bank; one act drains it
        ptr = C.pss((128, TG), BF16)
        for db in range(4):
            nc.tensor.transpose(ptr[:, db * 128:(db + 1) * 128],
                                yf[:, db * 128:(db + 1) * 128], C.identB[:])
        dst = yfd[:].rearrange("p (db t) -> p db t", db=4)[:, :, tt * 128:(tt + 1) * 128]
        src = ptr[:].rearrange("p (db t) -> p db t", db=4)
        nc.scalar.activation(dst, src, AF.Copy)

    nxs = []
    for mo in range(2):
        po = C.ps()
        for db in range(4):
            nc.tensor.matmul(po[:], W["Wout"][db][:, mo * 128:(mo + 1) * 128],
                             yfd[:, db * TG:(db + 1) * TG],
                             start=(db == 0), stop=False)
        nc.tensor.matmul(po[:], C.identR[:], xs[mo][:], start=False, stop=True)
        nx = C.xt()
        nc.scalar.activation(nx[:], po[:], AF.Copy)
        nxs.append(nx)
    return nxs


def emit_load_group(C, D, g):
    nc = C.nc
    xs = [C.xt() for _ in range(2)]
    for tt in range(NT):
        xT = C.ts((128, DIM))
        nc.sync.dma_start(xT[:], D["dtok"][(4 * g + tt) * 128:(4 * g + tt + 1) * 128, :])
        for i in range(2):
            ptr = C.pss((128, 128))
            nc.tensor.transpose(ptr[:], xT[:, i * 128:(i + 1) * 128], C.ident[:])
            nc.scalar.activation(xs[i][:, tt * 128:(tt + 1) * 128], ptr[:], AF.Copy)
    return xs


def emit_gate(C, fts, bts, gW):
    nc = C.nc
    merged = []
    for mo in range(2):
        pg = C.ps()
        ins = [fts[0], fts[1], bts[0], bts[1]]
        for kb in range(4):
            nc.tensor.matmul(pg[:], gW[kb][:, mo * 128:(mo + 1) * 128], ins[kb][:],
                             start=(kb == 0), stop=(kb == 3))
        ge = C.tf()
        nc.scalar.activation(ge[:], pg[:], AF.Exp, scale=-1.0)
        gsp = C.tf()
        nc.scalar.activation(gsp[:], ge[:], AF.Ln, bias=1.0)
        gate = C.tf()
        nc.scalar.activation(gate[:], gsp[:], AF.Exp, scale=-1.0)
        d = C.tf()
        nc.vector.tensor_tensor(d[:], fts[mo][:].bitcast(F32), bts[mo][:].bitcast(F32),
                                OP.subtract)
        m1 = C.tf()
        nc.vector.tensor_tensor(m1[:], gate[:], d[:], OP.mult)
        mg = C.mt()
        nc.vector.tensor_tensor(mg[:], m1[:], bts[mo][:].bitcast(F32), OP.add)
        merged.append(mg)
    return merged


_TABLES_PATCHED = False


def _pin_act_table():
    # Force every ACT instruction onto natural_log_exp_and_others (covers our
    # Exp/Ln/Square/Copy/Identity) so bacc never inserts per-function table
    # reloads (~1.3us each).
    global _TABLES_PATCHED
    if _TABLES_PATCHED:
        return
    import concourse.bacc as _bacc
    _orig = _bacc.get_activation_tables

    def _pinned(arch):
        t = _orig(arch)
        return {k: (v if k == "natural_log_exp_and_others" else set())
                for k, v in t.items()}

    _bacc.get_activation_tables = _pinned
    _TABLES_PATCHED = True


def build_nc():
    """Build the full SPMD program (uniform across cores)."""
    _pin_act_table()
    nc = bacc.Bacc(trn_type="TRN2", target_bir_lowering=False, debug=False,
                   enable_asserts=False, num_devices=8)
    epst = nc.alloc_sbuf_tensor("const-eps", [128, 1], F32)
    nc.gpsimd.memset(epst.ap(), EPS)
    nc.const_aps.aps[(F32, EPS)] = epst.ap()
    nc.all_engine_barrier()
    D = {}

    def inp(name, shape, dtype=F32R):
        D[name] = nc.dram_tensor(name, list(shape), dtype, kind="ExternalInput").ap()

    inp("dtok", (NTOK, DIM), F32)
    inp("Wx", (2, NL, DIM, DI)); inp("Wz", (2, NL, DIM, DI)); inp("Wdt", (2, NL, DIM, DI))
    inp("WB", (2, NL, DIM, ST)); inp("WC", (2, NL, DIM, ST))
    inp("Wout", (2, NL, DI, DIM))
    inp("dt_bias", (2, NL, 1, DI))
    inp("A_bc", (2, NL, 128, DI), F32)
    inp("gate_W", (DI, DIM))
    inp("merge_W", (2, DI, DIM), BF16)
    inp("ones128", (128, 128)); inp("ones_row", (1, 128))
    inp("ident", (128, 128), F32); inp("identR", (128, 128))
    inp("identB", (128, 128), BF16)
    inp("ltri_f", (128, 128)); inp("ltri_b", (128, 128))
    inp("mask_f", (128, ST), F32); inp("mask_b", (128, ST), F32)
    inp("csum_f", (128, ST)); inp("csum_b", (128, ST))
    inp("vmask", (128, 1), F32)

    osem = nc.dram_tensor("osem", [SH * SW, DIM], F32, kind="ExternalOutput").ap()
    oinst = nc.dram_tensor("oinst", [SH * SW, DIM], F32, kind="ExternalOutput").ap()
    fsp = nc.dram_tensor("fsp", [16, 128, TG], F32R).ap()

    with tile.TileContext(nc) as tc, ExitStack() as ctx:
        C = KC(nc, tc, ctx)
        cp = C.pool("consts", 1)

        def cload(name, shape, dtype=F32R):
            t = cp.tile(list(shape), dtype, tag=name, name=name)
            nc.sync.dma_start(t[:], D[name][:])
            return t

        C.ones128 = cload("ones128", (128, 128))
        C.ones_row = cload("ones_row", (1, 128))
        C.ident = cload("ident", (128, 128), F32)
        C.identB = cload("identB", (128, 128), BF16)
        C.identR = cload("identR", (128, 128))
        C.ltri_f = cload("ltri_f", (128, 128))
        C.ltri_b = cload("ltri_b", (128, 128))
        C.mask_f = cload("mask_f", (128, ST), F32)
        C.mask_b = cload("mask_b", (128, ST), F32)
        C.csum_f = cload("csum_f", (128, ST))
        C.csum_b = cload("csum_b", (128, ST))
        C.vmask = cload("vmask", (128, 1), F32)

        _build_full(C, D, osem, oinst, fsp)

    nc.compile()
    return nc


def _load_gate_w(C, D):
    nc = C.nc
    gW = []
    for kb in range(4):
        t = C.wt(f"gW{kb}", [128, DIM])
        nc.sync.dma_start(t[:], D["gate_W"][kb * 128:(kb + 1) * 128, :])
        gW.append(t)
    return gW


def _build_full(C, D, osem, oinst, fsp):
    nc = C.nc

    # self results (this core's direction), feature-major, group-major tokens
    Rt = [C.pool("Rt", 1).tile([128, 8 * TG], BF16, tag=f"Rt{k}", name=f"Rt{k}")
          for k in range(2)]
    # masked copy in ROW-MAJOR token order assuming v-scan source: real
    # permutation on v-cores, scrambled zeros on (masked) h-cores
    Rv = [C.pool("Rv", 1).tile([128, 8 * TG], BF16, tag=f"Rv{k}", name=f"Rv{k}")
          for k in range(2)]
    gW = _load_gate_w(C, D)

    dramp = C.pool("ccd", 1, space="DRAM")
    cc_in = [dramp.tile([256, 4 * TG], BF16, tag=f"cc_in{h}", name=f"cc_in{h}")
             for h in range(2)]
    cc_out = [dramp.tile([256, 4 * TG], BF16, tag=f"cc_out{h}", name=f"cc_out{h}")
              for h in range(2)]

    # ---------------- single-direction bicms ----------------
    for half in range(2):
        g0 = half * 4
        x_groups = [emit_load_group(C, D, g0 + g) for g in range(4)]
        for l in range(NL):
            W = emit_load_weights(C, D, 0, l)
            x_groups = [emit_layer(C, x_groups[g], W, True) for g in range(4)]
        for g in range(4):
            for i, t in enumerate(x_groups[g]):
                nc.sync.dma_start(fsp[2 * (g0 + g) + i, :, :], t[:])
        x_groups = [emit_load_group(C, D, g0 + g) for g in range(4)]
        W = emit_load_weights(C, D, 1, 0)
        x_groups = [emit_layer(C, x_groups[g], W, False) for g in range(4)]
        W = emit_load_weights(C, D, 1, 1)
        for g in range(4):
            gg = g0 + g
            bo = emit_layer(C, x_groups[g], W, False)
            f0 = C.tr(); nc.sync.dma_start(f0[:], fsp[2 * gg + 0, :, :])
            f1 = C.tr(); nc.sync.dma_start(f1[:], fsp[2 * gg + 1, :, :])
            mg = emit_gate(C, [f0, f1], bo, gW)
            for k in range(2):
                nc.scalar.activation(Rt[k][:, gg * TG:(gg + 1) * TG],
                                     mg[k][:].bitcast(F32), AF.Copy)
                # v-scan group gg covers cols w in [8*gg, 8*gg+8); scatter to
                # row-major flat (row*SW + w)*2 + s, scaled by the v-mask
                dstv = Rv[k][:].rearrange("p (h w two) -> p w h two",
                                          h=SH, w=SW, two=2)[:, 8 * gg:8 * gg + 8]
                srcv = mg[k][:].rearrange("p (wl h two) -> p wl h two",
                                          wl=8, h=SH, two=2)
                nc.scalar.activation(dstv, srcv, AF.Copy, scale=C.vmask[:])
        # exchange this half's columns while the next half computes
        for k in range(2):
            csrc = Rv[k][:].rearrange("p (h w two) -> p h w two",
                                      h=SH, w=SW, two=2)[:, :, 32 * half:32 * half + 32]
            nc.sync.dma_start(
                cc_in[half][k * 128:(k + 1) * 128, :].rearrange(
                    "p (h w two) -> p h w two", h=SH, w=32, two=2),
                csrc)
        nc.gpsimd.collective_compute(
            "AllReduce", OP.add,
            replica_groups=[[0, 1], [2, 3], [4, 5], [6, 7]],
            ins=[cc_in[half].opt()],
            outs=[cc_out[half].opt()],
        )



    # ---------------- merge: self-as-H + peer-as-V -------------------------
    Pv = [C.pool("Pv", 1).tile([128, 8 * TG], BF16, tag=f"Pv{k}", name=f"Pv{k}")
          for k in range(2)]
    for k in range(2):
        for h in range(2):
            dstp = Pv[k][:].rearrange("p (hh w two) -> p hh w two",
                                      hh=SH, w=SW, two=2)[:, :, 32 * h:32 * h + 32]
            nc.sync.dma_start(
                dstp,
                cc_out[h][k * 128:(k + 1) * 128, :].rearrange(
                    "p (hh w two) -> p hh w two", hh=SH, w=32, two=2))
    mW = {}
    for s in range(2):
        mW[s] = []
        for kb in range(4):
            t = C.wt(f"mW{s}{kb}", [128, DIM], BF16)
            nc.sync.dma_start(t[:], D["merge_W"][s, kb * 128:(kb + 1) * 128, :])
            mW[s].append(t)

    outd = {0: osem, 1: oinst}
    for gg in range(8):
        _emit_merge_tiles(C, Rt, Pv, gg, mW, outd)


def _emit_merge_tiles(C, Rt, Pv, g, mW, outd):
    nc = C.nc
    statg = C.pool("stat", 2).tile([128, 16], F32, tag="statg", name="statg")
    pms = {}
    for s in range(2):
        for r in range(2):
            i = 2 * g + r
            pm = C.ps((128, DIM))
            for k in range(2):
                srcp = Rt[k][:, g * TG:(g + 1) * TG].rearrange(
                    "p (rl w two) -> p rl w two",
                    rl=4, w=SW, two=2)[:, 2 * r:2 * r + 2, :, s]
                nc.tensor.matmul(pm[:], srcp, mW[s][k][:], start=(k == 0), stop=False)
            for k in range(2):
                srcv = Pv[k][:, i * 256:(i + 1) * 256].rearrange(
                    "p (t two) -> p t two", t=128, two=2)[:, :, s]
                nc.tensor.matmul(pm[:], srcv, mW[s][2 + k][:], start=False, stop=(k == 1))

            sc1 = C.ts((128, DIM))
            ci = (s * 2 + r) * 2
            nc.scalar.activation(sc1[:], pm[:], AF.Copy,
                                 accum_out=statg[:, ci:ci + 1])
            sc2 = C.ts((128, DIM))
            nc.scalar.activation(sc2[:], pm[:], AF.Square,
                                 accum_out=statg[:, ci + 1:ci + 2])
            pms[(s, r)] = sc1
    mu = C.pool("stat", 2).tile([128, 4], F32, tag="mu", name="mu")
    nc.scalar.activation(mu[:], statg[:, 0:8:2], AF.Copy, scale=1.0 / DIM)
    mu2 = C.pool("stat", 2).tile([128, 4], F32, tag="mu2", name="mu2")
    nc.scalar.activation(mu2[:], mu[:], AF.Square)
    var = C.pool("stat", 2).tile([128, 4], F32, tag="var", name="var")
    nc.vector.scalar_tensor_tensor(var[:], statg[:, 1:8:2], 1.0 / DIM, mu2[:],
                                   OP.mult, OP.subtract)
    sdt = C.pool("stat", 2).tile([128, 4], F32, tag="sdt", name="sdt")
    nc.scalar.activation(sdt[:], var[:], AF.Ln, bias=EPS)
    rs = C.pool("stat", 2).tile([128, 4], F32, tag="rs", name="rs")
    nc.scalar.activation(rs[:], sdt[:], AF.Exp, scale=-0.5)
    for s in range(2):
        for r in range(2):
            i = 2 * g + r
            ci = s * 2 + r
            sc1 = pms[(s, r)]
            t1 = C.ts((128, DIM))
            nc.vector.tensor_scalar(t1[:], sc1[:], mu[:, ci:ci + 1],
                                    rs[:, ci:ci + 1], OP.subtract, OP.mult)
            nc.sync.dma_start(outd[s][i * 128:(i + 1) * 128, :], t1[:])


# ---------------------------------------------------------------------------
# host side
# ---------------------------------------------------------------------------

_CACHE = {}


def _consts_np():
    import ml_dtypes
    q = 64
    tri = np.tril(np.ones((q, q), np.float32))          # tri[t, tau] t>=tau
    ltri_f = np.zeros((128, 128), np.float32)           # [tau, t] = tau<=t
    ltri_b = np.zeros((128, 128), np.float32)           # [tau, t] = tau>=t
    for c in range(2):
        ltri_f[c * q:(c + 1) * q, c * q:(c + 1) * q] = tri.T
        ltri_b[c * q:(c + 1) * q, c * q:(c + 1) * q] = tri
    mask_f = np.zeros((128, q), np.float32)
    mask_b = np.zeros((128, q), np.float32)
    for c in range(2):
        mask_f[c * q:(c + 1) * q, :] = tri.T
        mask_b[c * q:(c + 1) * q, :] = tri
    return {
        "ones128": np.ones((128, 128), np.float32),
        "ones_row": np.ones((1, 128), np.float32),
        "ident": np.eye(128, dtype=np.float32),
        "identB": np.eye(128).astype(ml_dtypes.bfloat16),
        "identR": np.eye(128, dtype=np.float32),
        "ltri_f": ltri_f, "ltri_b": ltri_b,
        "mask_f": mask_f, "mask_b": mask_b,
        "csum_f": np.concatenate([np.ones((64, 64), np.float32),
                                  np.zeros((64, 64), np.float32)]),
        "csum_b": np.concatenate([np.zeros((64, 64), np.float32),
                                  np.ones((64, 64), np.float32)]),
    }


def _check_trivial_params(inputs):
    """The compiled program folds away ops for parameters that are
    structurally trivial in this problem's setup_inputs (ln affine = identity,
    Dskip = 1, zero biases on gate/merge, identity merge LN affine). Verify
    that assumption on the actual inputs rather than trusting it silently."""
    f = lambda x: np.asarray(x, np.float32)
    checks = [
        (np.all(f(inputs["ln_s"]) == 1.0), "ln_s != 1"),
        (np.all(f(inputs["ln_b"]) == 0.0), "ln_b != 0"),
        (np.all(f(inputs["Dskip"]) == 1.0), "Dskip != 1"),
        (np.all(f(inputs["gate_b"]) == 0.0), "gate_b != 0"),
        (np.all(f(inputs["merge_b"]) == 0.0), "merge_b != 0"),
        (np.all(f(inputs["merge_ln_s"]) == 1.0), "merge_ln_s != 1"),
        (np.all(f(inputs["merge_ln_b"]) == 0.0), "merge_ln_b != 0"),
    ]
    bad = [msg for ok, msg in checks if not ok]
    if bad:
        raise NotImplementedError(
            f"kernel compiled for trivial affine/bias params; got {bad}")


def prep_inputs(inputs):
    """Per-core input maps: core 2b = horizontal bicms of image b, core
    2b+1 = vertical. Direction is encoded entirely in the data/weights."""
    import ml_dtypes
    f = lambda x: np.ascontiguousarray(np.asarray(x, np.float32))
    consts = _consts_np()
    Wfull = {n: f(inputs[n]) for n in ("Wx", "Wz", "Wdt", "WB", "WC", "Wout")}
    dt_bias = f(inputs["dt_bias"]).reshape(4, NL, 1, DI)
    A = -np.exp(f(inputs["A_log"]))
    A_bc = np.ascontiguousarray(np.broadcast_to(A[:, :, None, :], (4, NL, 128, DI)))
    gate_W = f(inputs["gate_W"])
    merge_W = f(inputs["merge_W"]).astype(ml_dtypes.bfloat16)
    csum_kill = np.full((128, ST), 1.0e4, np.float32)

    sem4 = f(inputs["stream_sem"]).reshape(BATCH, SH, SW, DIM)
    inst4 = f(inputs["stream_inst"]).reshape(BATCH, SH, SW, DIM)
    in_maps = []
    for c in range(8):
        b, p = c // 2, c % 2
        m = dict(consts)
        if p == 0:   # horizontal: row-major scan
            s2 = sem4[b].reshape(SH * SW, DIM)
            i2 = inst4[b].reshape(SH * SW, DIM)
        else:        # vertical: column-major scan
            s2 = np.ascontiguousarray(sem4[b].transpose(1, 0, 2)).reshape(SH * SW, DIM)
            i2 = np.ascontiguousarray(inst4[b].transpose(1, 0, 2)).reshape(SH * SW, DIM)
        m["dtok"] = np.ascontiguousarray(
            np.stack([s2, i2], axis=1).reshape(NTOK, DIM))
        sl = slice(2 * p, 2 * p + 2)   # h: stacks 0,1;  v: stacks 2,3
        for n in ("Wx", "Wz", "Wdt", "WB", "WC", "Wout"):
            m[n] = Wfull[n][sl]
        m["dt_bias"] = dt_bias[sl]
        m["A_bc"] = A_bc[sl]
        m["gate_W"] = gate_W[p]
        m["merge_W"] = merge_W
        if p == 1:
            m["csum_f"] = csum_kill
            m["csum_b"] = csum_kill
        m["vmask"] = np.full((128, 1), float(p), np.float32)
        in_maps.append(m)
    return in_maps


def kernel(**inputs):
    from concourse.bass_utils import run_bass_kernel_spmd
    _check_trivial_params(inputs)
    if "nc" not in _CACHE:
        _CACHE["nc"] = build_nc()
    nc = _CACHE["nc"]
    in_maps = prep_inputs(inputs)
    res = run_bass_kernel_spmd(nc, in_maps, list(range(8)))
    fused_sem = np.stack([res.results[2 * b]["osem"] for b in range(BATCH)])
    fused_inst = np.stack([res.results[2 * b]["oinst"] for b in range(BATCH)])
    return fused_sem, fused_inst


def timed_run(inputs, iters=6):
    """Steady-state device execution time per kernel invocation.

    A single synchronous dispatch through the axon/PJRT tunnel carries a
    ~74 ms fixed host<->terminal round trip (measured: a one-DMA trivial
    kernel times 73.7 ms), which swamps the actual on-device time. To
    measure hardware execution, enqueue K back-to-back executions
    asynchronously (PJRT serializes them per core) and take the marginal
    cost d(total)/dK between two batch sizes: the fixed RTT and dispatch
    pipeline cancel, leaving per-execution device time.
    """
    import time
    import jax
    import numpy as np_
    from jax.sharding import Mesh, PartitionSpec, NamedSharding
    from jax.experimental.shard_map import shard_map
    import concourse.mybir as mybir_
    from concourse import bass2jax

    if "nc" not in _CACHE:
        _CACHE["nc"] = build_nc()
    nc = _CACHE["nc"]
    in_maps = prep_inputs(inputs)
    n_cores = 8

    bass2jax.install_neuronx_cc_hook()
    partition_name = nc.partition_id_tensor.name if nc.partition_id_tensor else None
    in_names, out_names, out_avals, zero_outs = [], [], [], []
    for alloc in nc.m.functions[0].allocations:
        if not isinstance(alloc, mybir_.MemoryLocationSet):
            continue
        name = alloc.memorylocations[0].name
        if alloc.kind == "ExternalInput":
            if name != partition_name:
                in_names.append(name)
        elif alloc.kind == "ExternalOutput":
            shape = tuple(alloc.tensor_shape)
            dtype = mybir_.dt.np(alloc.dtype)
            out_names.append(name)
            out_avals.append(jax.core.ShapedArray(shape, dtype))
            zero_outs.append(np_.zeros(shape, dtype))
    n_params = len(in_names)
    n_outs = len(out_avals)
    all_in_names = list(in_names) + list(out_names)
    if partition_name is not None:
        all_in_names.append(partition_name)

    def _body(*args):
        operands = list(args)
        if partition_name is not None:
            operands.append(bass2jax.partition_id_tensor())
        return tuple(bass2jax._bass_exec_p.bind(
            *operands, out_avals=tuple(out_avals), in_names=tuple(all_in_names),
            out_names=tuple(out_names), lowering_input_output_aliases=(),
            sim_require_finite=True, sim_require_nnan=True, nc=nc))

    devices = jax.devices()[:n_cores]
    mesh = Mesh(np_.asarray(devices), ("core",))
    spec = PartitionSpec("core")
    fn = jax.jit(
        shard_map(_body, mesh=mesh, in_specs=(spec,) * (n_params + n_outs),
                  out_specs=(spec,) * n_outs, check_rep=False),
        keep_unused=True)
    sh = NamedSharding(mesh, spec)
    dev_in = [jax.device_put(
        np_.concatenate([np_.asarray(in_maps[c][nm]) for c in range(n_cores)], 0), sh)
        for nm in in_names]
    zs = [jax.device_put(np_.concatenate([z] * n_cores, 0), sh) for z in zero_outs]
    outs = fn(*dev_in, *zs)  # warmup (compile + first dispatch)
    jax.block_until_ready(outs)

    def batch_time(k):
        best = None
        for _ in range(max(2, iters // 2)):
            t0 = time.perf_counter()
            outs_list = [fn(*dev_in, *zs) for _ in range(k)]
            jax.block_until_ready(outs_list)
            dt = time.perf_counter() - t0
            best = dt if best is None else min(best, dt)
        return best

    k1, k2 = 4, 24
    t1, t2 = batch_time(k1), batch_time(k2)
    return int((t2 - t1) / (k2 - k1) * 1e9)
